# revision 32
# baseline (speedup 1.0000x reference)
"""CAM (channel attention module) Trainium2 Bass kernel.

Reference computation (per sample, x: [C, N] with N = H*W):
    energy    = x @ x.T                      # [C, C] Gram matrix
    att       = softmax(rowmax(energy) - energy, axis=-1)
              = softmax(-energy, axis=-1)    # identical after max-shift
    out       = att @ x                      # [C, N]
    result    = gamma * out + x

Sharding: data-parallel over batch, B=16 -> 2 samples per core on 8 cores.

Per-core dataflow (per sample):
  - x [256, 16384] stays resident in SBUF as 2x32 tiles of [128, 512],
    loaded once from HBM into float32r-typed tiles: the DGE rounds the
    payload to fp32r in flight, which satisfies walrus' requirement that
    every fp32r matmul operand come from a rounding producer.  Both the
    phase-1 transposes and the phase-2 moving operands then read the
    resident tiles directly -- no separate rounding-copy stage.  The +x
    residual reads the same tiles (rel err of the fp32r rounding ~1e-5,
    far below tolerance).  `prefetch` extra pool slots let the next
    sample's loads run during this sample's softmax boundary.
  - Phase 1 processes n-tiles in PAIRS: 4 PE transposes (fp32r, 1.5
    cyc/row) fill one [128, 512] PSUM tile, one wide eviction copy
    (alternating ScalarE/VectorE) moves it to SBUF, then two accumulating
    fp32r matmuls per tile build energy in a single [128, 512] PSUM bank.
  - Softmax: row-min shift (equivalent to the reference's max-shifted
    softmax); both row-min reduces are emitted before the exps so the
    in-order VectorE queue never stalls a reduce behind a reciprocal;
    exp on ScalarE with fused row-sum; 1/denom is folded into the
    phase-2 eviction scale; the two E^T evictions split ScalarE/VectorE.
  - Phase 2: out = E^T.T @ x with E^T stationary; eviction computes
    gamma/denom * psum + x in one VectorE op and streams to HBM.  Stores
    are emitted before the next sample's loads so a load blocked on a
    free SBUF slot never sits ahead of ready stores in the DMA FIFO.

HBM traffic is the floor: 16 MiB in + 16 MiB out per sample; the
cost-model DMA work is ~186 us per core.
"""

import threading

import numpy as np

import concourse.bass as bass
import concourse.mybir as mybir
import concourse.tile as tile
from concourse import bacc
from concourse.bass_utils import run_bass_kernel_spmd
from concourse.masks import make_identity

P = 128
F32 = mybir.dt.float32
F32R = mybir.dt.float32r

# Full-problem shapes (hardcoded per harness contract).
B_FULL = 16
C_FULL = 256
H_FULL = W_FULL = 128
N_CORES = 8
B_PER_CORE = B_FULL // N_CORES  # 2


def emit_cam(tc, x, gamma_b, out, n_s, C, N, xt_cols=512, chunk=512,
             prefetch=20, osb_bufs=6, ptr_bufs=3, pout_bufs=4,
             xft_bufs=6, interleave=True, p1_depth=3, xft_split=True):
    """Emit the per-core CAM kernel.

    x:       DRAM [n_s, C, N] f32
    gamma_b: DRAM [128, 1] f32 (gamma broadcast to all partitions on host)
    out:     DRAM [n_s, C, N] f32
    """
    nc = tc.nc
    cb_n = C // P            # channel blocks (2)
    nt = N // P              # n-tiles for transposes
    npair = nt // 2          # phase-1 pair steps
    nxt = N // xt_cols       # resident xf tiles per channel block
    nch = N // chunk         # phase-2 output chunks
    assert xt_cols % P == 0 and xt_cols == chunk and C == 256
    assert nt % 2 == 0 and npair % nch == 0 and nxt % nch == 0

    xf_bufs = 2 * nxt + prefetch
    with (
        tc.tile_pool(name="consts", bufs=1) as consts,
        tc.tile_pool(name="xf", bufs=xf_bufs) as xf_pool,
        tc.tile_pool(name="xft", bufs=xft_bufs) as xft_pool,
        tc.tile_pool(name="att", bufs=2) as att_pool,
        tc.tile_pool(name="attT", bufs=4) as attT_pool,
        tc.tile_pool(name="osb", bufs=osb_bufs) as osb_pool,
        tc.tile_pool(name="stat", bufs=4) as stat_pool,
        tc.tile_pool(name="eps", bufs=1, space="PSUM") as eps_pool,
        tc.tile_pool(name="ptr", bufs=ptr_bufs, space="PSUM") as ptr_pool,
        tc.tile_pool(name="pout", bufs=pout_bufs, space="PSUM") as pout_pool,
    ):
        # -------- per-sample stage emitters (state dict per sample) --------
        def new_state(s):
            return {"s": s, "xf": [[None] * nxt for _ in range(cb_n)],
                    "e_ps": None, "pend": [], "attT": None, "ginv": None}

        def emit_load(st, o):
            s = st["s"]
            for cb in range(cb_n):
                t_ = xf_pool.tile([P, xt_cols], F32R, tag="xf",
                                  name=f"xf_s{s}_c{cb}_o{o}")
                nc.sync.dma_start(
                    t_, x[s, cb * P:(cb + 1) * P, o * xt_cols:(o + 1) * xt_cols])
                st["xf"][cb][o] = t_

        # First x load is enqueued before the consts so the SDMA engines
        # start on real data immediately.
        st_first = new_state(0)
        emit_load(st_first, 0)

        identity = consts.tile([P, P], F32, tag="identity")
        make_identity(nc, identity)
        # fp32r identity for the phase-1 transposes, produced by a rounding
        # copy so walrus accepts it as an fp32r matmul operand
        identity_r = consts.tile([P, P], F32R, tag="identity_r")
        nc.scalar.copy(identity_r, identity)
        gamma_sb = consts.tile([P, 1], F32, tag="gamma")
        nc.gpsimd.dma_start(gamma_sb, gamma_b)

        def emit_tr(st, k):
            # Two n-tiles (2k, 2k+1) share one [128, 2C] PSUM tile so the
            # eviction is a single wide copy.
            s = st["s"]
            ptr = ptr_pool.tile([P, 2 * C], F32R, tag="ptr",
                                name=f"ptr_s{s}_k{k}")
            for half in range(2):
                t = 2 * k + half
                o, lc = divmod(t * P, xt_cols)
                for cb in range(cb_n):
                    nc.tensor.transpose(
                        ptr[:, half * C + cb * P:half * C + (cb + 1) * P],
                        st["xf"][cb][o][:, lc:lc + P], identity_r)
            xft = xft_pool.tile([P, 2 * C], F32R, tag="xft",
                                name=f"xft_s{s}_k{k}")
            if xft_split and k % 2 == 0:
                nc.vector.tensor_copy(xft, ptr)
            else:
                nc.scalar.copy(xft, ptr)
            return xft

        def emit_mm(st, k, xft):
            # energy lives in ONE [128, 2C] PSUM tile (both channel blocks
            # side by side) so it occupies a single PSUM bank
            for half in range(2):
                t = 2 * k + half
                base = half * C
                for mb in range(cb_n):
                    nc.tensor.matmul(
                        st["e_ps"][:, mb * C:(mb + 1) * C],
                        lhsT=xft[:, base + mb * P:base + (mb + 1) * P],
                        rhs=xft[:, base:base + C],
                        start=(t == 0), stop=(t == nt - 1))

        def p1_step(st, k):
            # software-pipelined at distance `p1_depth`: the matmuls of pair
            # k-p1_depth are emitted after the transposes of pair k, so the
            # PE always has transposes in hand while each pair's eviction
            # copy is in flight
            if st["e_ps"] is None:
                s = st["s"]
                st["e_ps"] = eps_pool.tile([P, 2 * C], F32, tag="eps",
                                           name=f"eps_s{s}")
            xft = emit_tr(st, k)
            st["pend"].append((k, xft))
            if len(st["pend"]) > p1_depth:
                emit_mm(st, *st["pend"].pop(0))

        def p1_flush(st):
            for pk in st["pend"]:
                emit_mm(st, *pk)
            st["pend"] = []

        def emit_softmax(st):
            # E = exp(rowmin - energy); denom = rowsum(E); then E^T tiles
            # (stationary operand of phase 2).  Both row-min reduces are
            # emitted before the exps: the in-order DVE queue would
            # otherwise stall reduce[1] behind recip[0] (which waits on
            # exp[0]'s accumulator).
            s = st["s"]
            ms = []
            for mb in range(cb_n):
                m = stat_pool.tile([P, 1], F32, tag="m", name=f"m_s{s}_{mb}")
                nc.vector.tensor_reduce(
                    m, st["e_ps"][:, mb * C:(mb + 1) * C],
                    axis=mybir.AxisListType.X, op=mybir.AluOpType.min)
                ms.append(m)
            att = []
            dens = []
            for mb in range(cb_n):
                a = att_pool.tile([P, C], F32, tag="att", name=f"att_s{s}_{mb}")
                den = stat_pool.tile([P, 1], F32, tag="den", name=f"den_s{s}_{mb}")
                nc.scalar.activation(
                    a, st["e_ps"][:, mb * C:(mb + 1) * C],
                    mybir.ActivationFunctionType.Exp,
                    bias=ms[mb], scale=-1.0, accum_out=den)
                att.append(a)
                dens.append(den)
            ginv = []
            for mb in range(cb_n):
                inv = stat_pool.tile([P, 1], F32, tag="inv", name=f"inv_s{s}_{mb}")
                nc.vector.reciprocal(inv, dens[mb])
                gi = stat_pool.tile([P, 1], F32, tag="gi", name=f"gi_s{s}_{mb}")
                nc.vector.tensor_tensor(gi, inv, gamma_sb, mybir.AluOpType.mult)
                ginv.append(gi)
            attT = []
            for jb in range(cb_n):
                ptr2 = ptr_pool.tile([P, C], F32, tag="ptr", name=f"ptrT_s{s}_{jb}")
                for ib in range(cb_n):
                    nc.tensor.transpose(
                        ptr2[:, ib * P:(ib + 1) * P],
                        att[ib][:, jb * P:(jb + 1) * P], identity)
                aT = attT_pool.tile([P, C], F32R, tag="attT",
                                    name=f"attT_s{s}_{jb}")
                # the two evictions run on different engines so they don't
                # serialize behind each other
                if jb == 0:
                    nc.scalar.copy(aT, ptr2)
                else:
                    nc.vector.tensor_copy(aT, ptr2)
                attT.append(aT)
            st["attT"] = attT
            st["ginv"] = ginv

        def p2_chunk(st, ch):
            # out = gamma/denom * (E^T.T @ x) + x for one 512-column chunk;
            # both the moving operand and the residual read the resident
            # fp32r x tiles directly
            s = st["s"]
            o, lc = divmod(ch * chunk, xt_cols)
            osb_dt = out.tensor.dtype
            for cb in range(cb_n):
                po = pout_pool.tile([P, chunk], F32, tag="pout",
                                    name=f"po_s{s}_c{ch}_{cb}")
                for jb in range(cb_n):
                    nc.tensor.matmul(
                        po,
                        lhsT=st["attT"][jb][:, cb * P:(cb + 1) * P],
                        rhs=st["xf"][jb][o][:, lc:lc + chunk],
                        start=(jb == 0), stop=(jb == cb_n - 1))
                osb = osb_pool.tile([P, chunk], osb_dt, tag="osb",
                                    name=f"osb_s{s}_c{ch}_{cb}")
                nc.vector.scalar_tensor_tensor(
                    osb, po, st["ginv"][cb],
                    st["xf"][cb][o][:, lc:lc + chunk].bitcast(F32),
                    op0=mybir.AluOpType.mult, op1=mybir.AluOpType.add)
                nc.sync.dma_start(
                    out[s, cb * P:(cb + 1) * P, ch * chunk:(ch + 1) * chunk], osb)

        # -------- schedule --------
        # Sample s's phase 2 is emitted interleaved with sample s+1's loads
        # and phase-1 pair steps, so the next sample's pipeline keeps pace
        # with its trickling loads instead of piling up a tail backlog.
        states = [st_first] + [new_state(s) for s in range(1, n_s)]
        st0 = states[0]
        for o in range(1, nxt):
            emit_load(st0, o)
        for k in range(npair):
            p1_step(st0, k)
        p1_flush(st0)
        emit_softmax(st0)
        for s in range(n_s):
            st = states[s]
            nxt_st = states[s + 1] if s + 1 < n_s else None
            if interleave and nxt_st is not None:
                pre_loads = prefetch // 2
                for o in range(pre_loads):
                    emit_load(nxt_st, o)
                next_pair = 0
                for ch in range(nch):
                    p2_chunk(st, ch)
                    if pre_loads + ch < nxt:
                        emit_load(nxt_st, pre_loads + ch)
                    # pair k reads tiles through o-block (2k+1)//4; emit up
                    # to 3 pairs per chunk among those already covered
                    avail = min(npair, 2 * (pre_loads + ch) + 2)
                    take = min(3, avail - next_pair)
                    for _ in range(max(0, take)):
                        p1_step(nxt_st, next_pair)
                        next_pair += 1
                while next_pair < npair:
                    p1_step(nxt_st, next_pair)
                    next_pair += 1
                p1_flush(nxt_st)
                emit_softmax(nxt_st)
            else:
                for ch in range(nch):
                    p2_chunk(st, ch)
                if nxt_st is not None:
                    for o in range(nxt):
                        emit_load(nxt_st, o)
                    for k in range(npair):
                        p1_step(nxt_st, k)
                    p1_flush(nxt_st)
                    emit_softmax(nxt_st)


def build_nc(n_s=B_PER_CORE, C=C_FULL, N=H_FULL * W_FULL, **kwargs):
    nc = bacc.Bacc("TRN2", target_bir_lowering=False, debug=False)
    # x is declared float32r: same 4-byte payload (dt.np(f32r)==float32, no
    # in-flight cast), but walrus then accepts the resident tiles as fp32r
    # matmul operands directly.  The PE rounds on consumption; the
    # residual bitcast-read stays bit-exact f32.
    x = nc.dram_tensor("x", [n_s, C, N], F32R, kind="ExternalInput").ap()
    gamma_b = nc.dram_tensor("gamma_b", [P, 1], F32, kind="ExternalInput").ap()
    out = nc.dram_tensor("out", [n_s, C, N], F32, kind="ExternalOutput").ap()
    with tile.TileContext(nc) as tc:
        emit_cam(tc, x, gamma_b, out, n_s, C, N, **kwargs)
    nc.compile()
    return nc


_CACHE = threading.Lock()
_NC = None


def _get_nc():
    global _NC
    with _CACHE:
        if _NC is None:
            _NC = build_nc()
    return _NC


def run_spmd(x, gamma, **kwargs):
    """Shard inputs over 8 cores, run, gather. Returns (output, BassKernelResults)."""
    x = np.ascontiguousarray(np.asarray(x), dtype=np.float32)
    assert x.shape == (B_FULL, C_FULL, H_FULL, W_FULL), x.shape
    n = H_FULL * W_FULL
    xs = x.reshape(B_FULL, C_FULL, n)
    gb = np.full((P, 1), np.float32(np.asarray(gamma)), dtype=np.float32)
    in_maps = [
        {"x": xs[c * B_PER_CORE:(c + 1) * B_PER_CORE], "gamma_b": gb}
        for c in range(N_CORES)
    ]
    nc = _get_nc()
    res = run_bass_kernel_spmd(nc, in_maps, core_ids=list(range(N_CORES)), **kwargs)
    outs = np.stack([res.results[c]["out"] for c in range(N_CORES)])
    full = outs.reshape(B_FULL, C_FULL, H_FULL, W_FULL).astype(np.float32, copy=False)
    return full, res


def kernel(x, gamma):
    out, _ = run_spmd(x, gamma)
    return out
